# revision 37
# baseline (speedup 1.0000x reference)
"""Trainium2 Bass kernel for nn_AV_MiltiHeadAttention.

Strategy: data-parallel over B across 8 NeuronCores (1 batch element per core).
Per-core everything is kept in a "transposed" (feature-on-partitions) layout so
no on-chip transposes are needed:

  audiaT/lipT/W*T are marshalled on host (transpose + bf16 cast = input
  layout prep for the SPMD shards), all matmul contractions then have their
  contraction dim on SBUF partitions naturally.

  S.T[m,n] = sum_d kTz[d,m] qT[d,n]         (zero-padded per head: K=128, so
                                             every matmul shares one PE tiling
                                             mode -> no array-drain switches)
  E = exp(S.T * scale)                       (no max subtraction; |S*scale|<2)
  raw[j,n] = sum_m [k_h | 1][m,j] E[m,n]    -> rows 0..63 = attn_qk.T (unnorm),
                                               row 64 = softmax denominator r
  1/r, 1/s chains on [1,N] rows (DVE recip; partition-0 constraint), placed
  at rows 0/64 of zeroed [65,N] tiles; K=65 selector matmuls broadcast them
  across all 128 partitions in PSUM (65 rounds to row-size 128: no mode
  switch), so each normalize is a single [128,N] tensor_tensor.
  y = attn_qk.T * lip_v.T ; expY = exp(y)
  s = colsum_d(expY) via [128,65] selector matmul (softmax over head_dim)
  z.T = attn_qk.T * expY * (1/s)
  av_attn = sigmoid(2 * qs.T@qs / temp)      (SE layer)
  Wc = W_proj.T @ av_attn ; row = b_proj @ av_attn   (proj/out fusion)
  out = z @ Wc + row                          (single fused output GEMM; bias
                                             and cb<3 partials accumulate
                                             early, only cb=3 + add + DMA
                                             remain after the last z lands)

All matmuls bf16 x bf16 -> fp32 PSUM. Softmax normalization in fp32.
"""

import os
import numpy as np
import ml_dtypes

DEBUG_DUMP = os.environ.get("KDBG", "0") == "1"

B, N, C = 8, 1024, 512
H, HD = 8, 64
CB = C // 128          # 4 chunks of the feature dim
MB = N // 128          # 8 chunks of the token dim
NH = N // 512          # 2 halves of the token dim (matmul free dim = 512)
SCALE = HD ** -0.5
TEMP = C ** 0.5

_CACHED = {}


def build_nc():
    import concourse.bass as bass
    import concourse.tile as tile
    import concourse.mybir as mybir
    from concourse import bacc
    from contextlib import ExitStack

    f32 = mybir.dt.float32
    bf16 = mybir.dt.bfloat16
    AF = mybir.ActivationFunctionType
    MUL = mybir.AluOpType.mult
    ADD = mybir.AluOpType.add

    nc = bacc.Bacc("TRN2", target_bir_lowering=False, debug=False, num_devices=B)

    d_audiaT = nc.dram_tensor("audiaT", [C, N], bf16, kind="ExternalInput")
    d_lipT = nc.dram_tensor("lipT", [C, N], bf16, kind="ExternalInput")
    d_WqkT = nc.dram_tensor("WqkT", [C, 2 * C], bf16, kind="ExternalInput")
    d_WlipT = nc.dram_tensor("WlipT", [C, C], bf16, kind="ExternalInput")
    d_WseT = nc.dram_tensor("WseT", [C, C], bf16, kind="ExternalInput")
    d_WprojN = nc.dram_tensor("WprojN", [C, C], bf16, kind="ExternalInput")
    d_bprojP = nc.dram_tensor("bprojP", [128, CB, HD + 1], bf16, kind="ExternalInput")
    d_ident = nc.dram_tensor("ident", [128, 128], bf16, kind="ExternalInput")
    d_sel = nc.dram_tensor("sel", [128, HD + 1], bf16, kind="ExternalInput")
    d_selC = nc.dram_tensor("selC", [HD + 1, 128], bf16, kind="ExternalInput")
    d_onesK = nc.dram_tensor("onesK", [HD + 1, 128], bf16, kind="ExternalInput")
    d_out = nc.dram_tensor("out", [N, C], bf16, kind="ExternalOutput")
    dbg = {}
    if DEBUG_DUMP:
        for nm, shp, dt in [
            ("dbg_qT", [128, CB, N], bf16), ("dbg_kTz", [128, CB, 2, N], bf16),
            ("dbg_kaug", [128, MB, H * (HD + 1)], bf16),
            ("dbg_lipv", [128, CB, N], bf16), ("dbg_qs", [128, MB, C], bf16),
            ("dbg_E0", [128, MB, 2 * N], bf16),
            ("dbg_rawsb0", [HD, N], bf16), ("dbg_attn0", [128, N], bf16),
            ("dbg_r00", [1, N], f32),
            ("dbg_y0", [128, N], bf16), ("dbg_expY0", [128, N], bf16),
            ("dbg_sblo0", [HD, N], f32), ("dbg_sbhi0", [HD, N], f32),
            ("dbg_zT", [128, CB, N], bf16), ("dbg_avattn", [128, CB, C], bf16),
            ("dbg_wc", [128, CB, C], bf16),
        ]:
            dbg[nm] = nc.dram_tensor(nm, shp, dt, kind="ExternalOutput")

    with tile.TileContext(nc) as tc, ExitStack() as ctx:
        persist = ctx.enter_context(tc.tile_pool(name="persist", bufs=1))
        # psA: 2x [128,1024] fp32 (2 banks each) for the S/qkT/partial/final
        # streams; psB: 4x [128,512] (1 bank each) -- fine-grained ring for
        # the raw/normalize chains so a held tile blocks less of the ring.
        psA = ctx.enter_context(tc.tile_pool(name="psA", bufs=2, space="PSUM"))
        psB = ctx.enter_context(tc.tile_pool(name="psB", bufs=4, space="PSUM"))

        # ---- persistent SBUF tensors ----
        qT = persist.tile([128, CB, N], bf16, tag="qT")        # q.T  [(h,d), n]
        # k.T zero-padded per head so the S matmuls run K=128 (no PE
        # tiling-mode switches): chunk 0 = [k_h_lo; 0], chunk 1 = [0; k_h_hi]
        # (measured: 64-row tiling loses ~18us to mode-switch drains because
        # the scheduler interleaves S with 128-mode matmuls, and the pairs
        # never co-stream -- the psA ring is ACT-gated.  Zeros via the idle
        # GpSimd engine, off the DVE.)
        kTz = persist.tile([128, CB, 2, N], bf16, tag="kTz")
        k_aug = persist.tile([128, MB, H * (HD + 1)], bf16, tag="k_aug")
        lip_vT = persist.tile([128, CB, N], bf16, tag="lip_vT")
        qs = persist.tile([128, MB, C], bf16, tag="qs")        # qs natural [n, c]
        WprojN_sb = persist.tile([128, CB, C], bf16, tag="WprojN_sb")
        Wc_sb = persist.tile([128, CB, C], bf16, tag="Wc_sb")
        bprojP_sb = persist.tile([128, CB, HD + 1], bf16, tag="bprojP_sb")
        ident_sb = persist.tile([128, 128], bf16, tag="ident_sb")
        sel_sb = persist.tile([128, HD + 1], bf16, tag="sel_sb")
        zT = persist.tile([128, CB, N], bf16, tag="zT")
        av_attn = persist.tile([128, CB, C], bf16, tag="av_attn")
        # bias row for the output GEMM, as K=65 rhs (rows 1:64 zeroed so the
        # onesK zero-rows multiply clean values, never junk NaNs)
        row_bf = persist.tile([HD + 1, C], bf16, tag="row_bf")
        nc.gpsimd.memset(row_bf[:], 0.0)
        # K=65 broadcast stationaries (65 rounds to tile row size 128, so no
        # PE tiling-mode switches): selC row 0 -> cols 0:64, row 64 -> cols
        # 64:128; onesK row 0 -> all cols.  rhs tiles r2/s2 hold per-head
        # reciprocal rows at partitions 0 and 64, zeroed elsewhere; two
        # alternating copies avoid cross-p WAR stalls.
        # the tiny selector tiles are fetched first, at the head of the
        # gpsimd queue, so the PE warm-up spin below can start ~8us before
        # the big audiaT/WqkT transfers land
        selC = persist.tile([HD + 1, 128], bf16, tag="selC")
        onesK = persist.tile([HD + 1, 128], bf16, tag="onesK")
        nc.gpsimd.dma_start(selC[:], d_selC[:])
        nc.gpsimd.dma_start(onesK[:], d_onesK[:])
        # kTz zero-padding memsets ride the gpsimd queue after the two tiny
        # DMA submits (the S(0) matmuls that read them start at ~17us)
        nc.gpsimd.memset(kTz[64:128, :, 0, :], 0.0)
        nc.gpsimd.memset(kTz[0:64, :, 1, :], 0.0)
        r2s = [persist.tile([HD + 1, N], bf16, tag=f"r2_{i}", name=f"r2_{i}")
               for i in range(2)]
        s2s = [persist.tile([HD + 1, N], bf16, tag=f"s2_{i}", name=f"s2_{i}")
               for i in range(2)]
        for t in r2s + s2s:
            nc.gpsimd.memset(t[:], 0.0)

        attn = ctx.enter_context(tc.tile_pool(name="attn", bufs=1))
        early_ctx = tc.tile_pool(name="early", bufs=1)
        early = early_ctx.__enter__()
        audiaT_sb = early.tile([128, CB, N], bf16, tag="audiaT_sb")
        lipT_sb = early.tile([128, CB, N], bf16, tag="lipT_sb")
        WqkT_sb = early.tile([128, CB, 2 * C], bf16, tag="WqkT_sb")
        WlipT_sb = early.tile([128, CB, C], bf16, tag="WlipT_sb")
        WseT_sb = early.tile([128, CB, C], bf16, tag="WseT_sb")

        # input DMAs ordered by first use and spread across engine DMA rings:
        # qkT needs WqkT(q half) [sync ring] + audiaT [scalar ring] first;
        # the k half [vector ring] feeds emit_kaug; lip/se/proj follow.
        d_WqkT_r = d_WqkT[:].rearrange("(cb p) c -> cb p c", p=128)
        d_audiaT_r = d_audiaT[:].rearrange("(cb p) n -> cb p n", p=128)
        for cb in range(CB):
            nc.sync.dma_start(WqkT_sb[:, cb, 0:C], d_WqkT_r[cb][:, 0:C])
            nc.scalar.dma_start(audiaT_sb[:, cb], d_audiaT_r[cb])
        for cb in range(CB):
            nc.sync.dma_start(WqkT_sb[:, cb, C:2 * C], d_WqkT_r[cb][:, C:2 * C])
        nc.scalar.dma_start(WlipT_sb[:], d_WlipT[:].rearrange("(cb p) c -> p cb c", p=128))
        nc.scalar.dma_start(lipT_sb[:], d_lipT[:].rearrange("(cb p) n -> p cb n", p=128))
        nc.sync.dma_start(sel_sb[:], d_sel[:])
        nc.sync.dma_start(WseT_sb[:], d_WseT[:].rearrange("(cb p) c -> p cb c", p=128))
        nc.scalar.dma_start(WprojN_sb[:], d_WprojN[:].rearrange("(cb p) c -> p cb c", p=128))
        nc.scalar.dma_start(bprojP_sb[:], d_bprojP[:])
        nc.gpsimd.dma_start(ident_sb[:], d_ident[:])

        # ---- PE warm-up: the HAM clock gate keeps the PE at 1.2 GHz until
        # it has seen ~3.4us of sustained matmul activity.  The first real
        # matmul can't start until audiaT/WqkT land (~10us of DMA submit +
        # transfer), so spin small matmuls on the tiny selector tiles (which
        # land in ~1us via the idle vector ring): qkT then opens at 2.4 GHz.
        # Results are never read.
        warm = psB.tile([128, 512], f32, tag="psB", name="warm_ps")
        for _ in range(26):
            nc.tensor.matmul(warm[:, 0:128], selC[:], onesK[:, 0:128],
                             start=True, stop=True)

        # ---- P1 pieces (emitted interleaved with attention below) ----
        def emit_qkT():
            # qT / kTz: out [oc:128, n] ; lhsT = WqkT block, rhs = audiaT
            for dst, oc0 in ((qT, 0), (kTz, C)):
                for ocb in range(CB):
                    ps = psA.tile([128, N], f32, tag="psA")
                    for cb in range(CB):
                        for nh in range(NH):
                            nc.tensor.matmul(
                                ps[:, nh * 512:(nh + 1) * 512],
                                WqkT_sb[:, cb, oc0 + ocb * 128: oc0 + (ocb + 1) * 128],
                                audiaT_sb[:, cb, nh * 512:(nh + 1) * 512],
                                start=(cb == 0), stop=(cb == CB - 1),
                            )
                    if dst is qT:
                        nc.vector.tensor_copy(out=dst[:, ocb, :], in_=ps[:])
                    else:
                        nc.vector.tensor_copy(out=kTz[0:64, ocb, 0, :], in_=ps[0:64, :])
                        nc.scalar.activation(kTz[64:128, ocb, 1, :], ps[64:128, :], AF.Copy)

        def emit_kaug():
            # k natural (+ trailing ones column: raw row 64 = softmax denom r)
            for mb in range(MB):
                ps = psB.tile([128, 512], f32, tag="psB")
                for cb in range(CB):
                    nc.tensor.matmul(
                        ps[:],
                        audiaT_sb[:, cb, mb * 128:(mb + 1) * 128],
                        WqkT_sb[:, cb, C:2 * C],
                        start=(cb == 0), stop=(cb == CB - 1),
                    )
                nc.vector.tensor_copy(
                    out=k_aug[:, mb].rearrange("p (h e) -> p h e", e=HD + 1)[:, :, 0:HD],
                    in_=ps[:].rearrange("p (h d) -> p h d", d=HD),
                )
            nc.vector.memset(
                k_aug[:].rearrange("p m (h e) -> p m h e", e=HD + 1)[:, :, :, HD:HD + 1], 1.0
            )

        def emit_lipv():
            for ocb in range(CB):
                pss = [psB.tile([128, 512], f32, tag="psB", name=f"lipv_ps{ocb}_{i}")
                       for i in range(NH)]
                for cb in range(CB):
                    for nh in range(NH):
                        nc.tensor.matmul(
                            pss[nh][:],
                            WlipT_sb[:, cb, ocb * 128:(ocb + 1) * 128],
                            lipT_sb[:, cb, nh * 512:(nh + 1) * 512],
                            start=(cb == 0), stop=(cb == CB - 1),
                        )
                for nh in range(NH):
                    nc.vector.tensor_copy(
                        out=lip_vT[:, ocb, nh * 512:(nh + 1) * 512], in_=pss[nh][:]
                    )

        def emit_qs():
            # qs natural [n, c] (needed on both sides of the SE bmm)
            for nb in range(MB):
                ps = psB.tile([128, 512], f32, tag="psB")
                for cb in range(CB):
                    nc.tensor.matmul(
                        ps[:],
                        audiaT_sb[:, cb, nb * 128:(nb + 1) * 128],
                        WseT_sb[:, cb, :],
                        start=(cb == 0), stop=(cb == CB - 1),
                    )
                nc.vector.tensor_copy(out=qs[:, nb, :], in_=ps[:])

        # ---- attention + SE ----
        def emit_se():
            # x[c,d] = sum_n qs[n,c] qs[n,d];  av_attn = sigmoid(2x/temp)
            # computed as 0.5 + 0.5*tanh(x/temp): tanh lives in the same ACT
            # table set as exp, so no mid-kernel ACT_TABLE_LOAD thrash.
            for cb in range(CB):
                ps = psB.tile([128, 512], f32, tag="psB")
                for nb in range(MB):
                    nc.tensor.matmul(
                        ps[:],
                        qs[:, nb, cb * 128:(cb + 1) * 128],
                        qs[:, nb, :],
                        start=(nb == 0), stop=(nb == MB - 1),
                    )
                th = attn.tile([128, C], bf16, tag="th", bufs=2, name=f"th{cb}")
                nc.scalar.activation(th[:], ps[:], AF.Tanh, scale=1.0 / TEMP)
                nc.vector.tensor_scalar(av_attn[:, cb, :], th[:], 0.5, 0.5, MUL, ADD)

        def emit_wc():
            # Wc = W_proj.T @ av_attn  (fold the proj GEMM into the output GEMM)
            for ccb in range(CB):
                ps = psA.tile([128, N], f32, tag="psA")
                for eb in range(CB):
                    nc.tensor.matmul(
                        ps[:, 0:512],
                        WprojN_sb[:, eb, ccb * 128:(ccb + 1) * 128],
                        av_attn[:, eb, :],
                        start=(eb == 0), stop=(eb == CB - 1),
                    )
                nc.scalar.activation(Wc_sb[:, ccb, :], ps[:, 0:512], AF.Copy)
            # row = b_proj @ av_attn (M=65 zero-padded stationary: no tiling
            # switch); prefolded into the partial-output psum chains via a
            # K=65 onesK matmul.
            rp = psB.tile([128, 512], f32, tag="psB")
            for cb in range(CB):
                nc.tensor.matmul(
                    rp[0:HD + 1, :],
                    bprojP_sb[:, cb, :],
                    av_attn[:, cb, :],
                    start=(cb == 0), stop=(cb == CB - 1),
                )
            nc.vector.tensor_copy(out=row_bf[0:1, :], in_=rp[0:1, :])
            if DEBUG_DUMP:
                nc.sync.dma_start(dbg["dbg_wc"][:], Wc_sb[:])

        def emit_S(p):
            # E = exp(S.T * scale) for heads (2p, 2p+1)
            E = attn.tile([128, MB, 2 * N], bf16, tag="E", bufs=2, name=f"E{p}")
            for mb in range(MB):
                for hh in range(2):
                    ps = psA.tile([128, N], f32, tag="psA")
                    for nh in range(NH):
                        nc.tensor.matmul(
                            ps[:, nh * 512:(nh + 1) * 512],
                            kTz[:, p, hh, mb * 128:(mb + 1) * 128],
                            qT[:, p, nh * 512:(nh + 1) * 512],
                            start=True, stop=True,
                        )
                    nc.scalar.activation(
                        E[:, mb, hh * N:(hh + 1) * N], ps[:], AF.Exp, scale=SCALE
                    )
            return E

        def emit_qk_front(p, E):
            # raw = [k_h|1].T @ E_h for BOTH heads back-to-back (keeps the PE
            # dense so HAM stays at full clock), then the first-softmax
            # normalize chain through y/expY.  The s-path + z live in
            # emit_qk_back so PE filler work can be emitted between them
            # (the PE queue is in-order: a stalled s_ps matmul would block
            # every filler emitted after it).
            attn_sb = attn.tile([128, N], bf16, tag="attn_sb", bufs=2, name=f"attn_sb{p}")
            raw2 = attn.tile([128, N], bf16, tag="raw_sb", bufs=2, name=f"raw2_{p}")
            for hh in range(2):
                h = 2 * p + hh
                raws = [psB.tile([128, 512], f32, tag="psB", name=f"raw{p}_{hh}_{i}")
                        for i in range(NH)]
                for mb in range(MB):
                    for nh in range(NH):
                        nc.tensor.matmul(
                            raws[nh][0:HD + 1, :],
                            k_aug[:, mb, h * (HD + 1):(h + 1) * (HD + 1)],
                            E[:, mb, hh * N + nh * 512: hh * N + (nh + 1) * 512],
                            start=(mb == 0), stop=(mb == MB - 1),
                        )
                # psum->sbuf copies: both heads' attn rows pack into one
                # [128,N] tile (partition-shifted write for the hi head),
                # and the raw denominator row r (psum row 64) drops straight
                # into the selC rhs r2 (bf16).  The r copies go first: the
                # broadcast matmul waits on them, the attn TT comes later.
                # The broadcast carries r (not 1/r); the reciprocal runs
                # full-width in PSUM after, so no [1,N] lane-starved chains.
                # For p>=2 the ACT queue has drained its S exps, so the
                # lo-head ops ride ACT for free.
                r2 = r2s[p % 2]
                for nh in range(NH):
                    sl = slice(nh * 512, (nh + 1) * 512)
                    if p >= 2 and hh == 0:
                        nc.scalar.activation(r2[0:1, sl], raws[nh][HD:HD + 1, :],
                                             AF.Copy)
                    else:
                        nc.vector.tensor_copy(
                            out=r2[hh * 64:hh * 64 + 1, sl],
                            in_=raws[nh][HD:HD + 1, :],
                        )
                for nh in range(NH):
                    sl = slice(nh * 512, (nh + 1) * 512)
                    if p >= 2 and hh == 0:
                        nc.scalar.activation(raw2[0:64, sl], raws[nh][0:HD, :],
                                             AF.Copy)
                    else:
                        nc.vector.tensor_copy(
                            out=raw2[hh * 64:hh * 64 + 64, sl], in_=raws[nh][0:HD, :]
                        )
            # K=65 selector matmul broadcasts r_lo to rows 0:64 and r_hi to
            # rows 64:128 of one PSUM tile; a full-width in-place reciprocal
            # then yields 1/r on all 128 partitions -> single normalize TT.
            # The whole chain runs at nh-half granularity so downstream
            # stages (and ultimately the output finals) start early.
            if p == 2:
                rbt = psA.tile([128, N], f32, tag="psA")
                rbps = [rbt[:, 0:512], rbt[:, 512:1024]]
            else:
                rbps = [psB.tile([128, 512], f32, tag="psB", name=f"rb{p}_{i}")[:]
                        for i in range(NH)]
            y = attn.tile([128, N], bf16, tag="y", bufs=2, name=f"y{p}")
            expY = attn.tile([128, N], bf16, tag="expY", bufs=2, name=f"expY{p}")
            for nh in range(NH):
                sl = slice(nh * 512, (nh + 1) * 512)
                nc.tensor.matmul(rbps[nh], selC[:], r2[:, sl],
                                 start=True, stop=True)
                nc.vector.reciprocal_approx_fast(out=rbps[nh], in_=rbps[nh])
                nc.vector.tensor_tensor(attn_sb[:, sl], raw2[:, sl], rbps[nh], MUL)
                # y gates expY -> the whole s-chain: keep it on the fast DVE
                # (a GpSimd TT takes 1.15us vs 0.33us and measured as the
                # largest late-phase PE gap when p==2 rode GpSimd).
                nc.vector.tensor_tensor(y[:, sl], attn_sb[:, sl],
                                        lip_vT[:, p, sl], MUL)
                nc.scalar.activation(expY[:, sl], y[:, sl], AF.Exp)
            if DEBUG_DUMP and p == 0:
                nc.sync.dma_start(dbg["dbg_rawsb0"][:], raw2[0:HD, :])
                nc.sync.dma_start(dbg["dbg_attn0"][:], attn_sb[:])
                nc.sync.dma_start(dbg["dbg_y0"][:], y[:])
                nc.sync.dma_start(dbg["dbg_expY0"][:], expY[:])
            return attn_sb, expY

        def emit_qk_back(p, attn_sb, expY):
            # s_lo lands at PSUM row 0, s_hi at row 64 (sel is [128,65] so
            # M=65 rounds to 128: no tiling-mode switch).  The whole s-path
            # runs at nh-half granularity so zT's first half lands early -
            # the output finals for nb 0..3 only need columns 0:512.
            # For p==2 park the psums on psA - they hold their buffers to
            # the end of the chain and would starve the psB ring the se/wc
            # fillers need.  (p==3 must stay on psB: psA holds the output
            # partials.)
            if p == 2:
                sbt = psA.tile([128, N], f32, tag="psA")
                sbps = [sbt[:, 0:512], sbt[:, 512:1024]]
            else:
                sbps = [psB.tile([128, 512], f32, tag="psB", name=f"sb{p}_{i}")[:]
                        for i in range(NH)]
            s2 = s2s[p % 2]
            u = attn.tile([128, N], bf16, tag="y", bufs=2, name=f"u{p}")
            for nh in range(NH):
                sl = slice(nh * 512, (nh + 1) * 512)
                s_ps = psB.tile([128, 512], f32, tag="psB", name=f"s_ps{p}_{nh}")
                nc.tensor.matmul(s_ps[0:HD + 1, :], sel_sb[:], expY[:, sl],
                                 start=True, stop=True)
                # s rows (psum rows 0/64) drop straight into the selC rhs s2
                # (bf16); the broadcast matmul carries s, and a full-width
                # in-place reciprocal in PSUM yields 1/s on all partitions.
                # The lo row is PSUM-partition-aligned (0->0) so its copy
                # rides the ACT engine once it has drained its S exps.
                for hh in range(2):
                    if p >= 2 and hh == 0:
                        nc.scalar.activation(s2[0:1, sl], s_ps[0:1, :], AF.Copy)
                    else:
                        nc.vector.tensor_copy(
                            out=s2[hh * 64:hh * 64 + 1, sl],
                            in_=s_ps[hh * 64:hh * 64 + 1, :],
                        )
                nc.tensor.matmul(sbps[nh], selC[:], s2[:, sl],
                                 start=True, stop=True)
                nc.vector.reciprocal_approx_fast(out=sbps[nh], in_=sbps[nh])
                # u is off the latency chain for p<2 (zT isn't needed until
                # the partials) -> GpSimd; p>=2 stays DVE (partials/finals
                # wait on zT 2/3, and a GpSimd TT costs 1.15us).
                if p <= 1:
                    nc.gpsimd.tensor_tensor(u[:, sl], attn_sb[:, sl], expY[:, sl], MUL)
                else:
                    nc.vector.tensor_tensor(u[:, sl], attn_sb[:, sl], expY[:, sl], MUL)
                nc.vector.tensor_tensor(zT[:, p, sl], u[:, sl], sbps[nh], MUL)

        # 2-deep software pipeline: S(p+1) overlaps qk(p); the independent
        # projection/SE matmuls are spread through the ACT-bound S windows
        # as PE filler.  se/wc fill qk(2)'s normalize window; the bias +
        # cb=0..2 partial output accumulation (evacuated to SBUF bf16 by the
        # idle ACT engine) fills qk(3)'s; only the cb=3 matmul + one add +
        # DMA remain after zT(3) lands.
        emit_qkT()
        E0 = emit_S(0)
        emit_kaug()
        E1 = emit_S(1)
        emit_lipv()
        fb0 = emit_qk_front(0, E0)
        if DEBUG_DUMP:
            nc.sync.dma_start(dbg["dbg_E0"][:], E0[:])
        E2 = emit_S(2)
        emit_qk_back(0, *fb0)
        emit_qs()
        early_ctx.__exit__(None, None, None)
        fb1 = emit_qk_front(1, E1)
        E3 = emit_S(3)
        emit_qk_back(1, *fb1)
        fb2 = emit_qk_front(2, E2)
        emit_se()
        emit_wc()
        emit_qk_back(2, *fb2)
        fb3 = emit_qk_front(3, E3)
        # bias + cb=0..2 partial output accumulation for all 8 nb chunks;
        # two nb chunks pack into each [128,1024] psA tile, evacuated to
        # partial_sb so the psA ring keeps cycling.
        late = ctx.enter_context(tc.tile_pool(name="late", bufs=1))
        partial_sb = late.tile([128, MB, C], bf16, tag="partial_sb")

        def emit_partials(ilo, ihi):
            for i in range(ilo, ihi):
                ps = psA.tile([128, N], f32, tag="psA")
                for half in range(2):
                    nb = 2 * i + half
                    nc.tensor.matmul(
                        ps[:, half * 512:(half + 1) * 512],
                        onesK[:], row_bf[:],
                        start=True, stop=False,
                    )
                    for cb in range(CB - 1):
                        nc.tensor.matmul(
                            ps[:, half * 512:(half + 1) * 512],
                            zT[:, cb, nb * 128:(nb + 1) * 128],
                            Wc_sb[:, cb, :],
                            start=False, stop=(cb == CB - 2),
                        )
                # alternate ACT/DVE per [128,512] quantum so neither queue
                # stalls a latency-critical s2/r2 copy behind bulk evac
                for half in range(2):
                    if (2 * i + half) % 2 == 0:
                        nc.scalar.activation(partial_sb[:, 2 * i + half, :],
                                             ps[:, half * 512:(half + 1) * 512],
                                             AF.Copy)
                    else:
                        nc.vector.tensor_copy(
                            out=partial_sb[:, 2 * i + half, :],
                            in_=ps[:, half * 512:(half + 1) * 512],
                        )

        # first half before back(3) so its s-path matmuls aren't queued
        # behind all 40 partial MMs; second half fills back(3)'s window
        emit_partials(0, 2)
        emit_qk_back(3, *fb3)
        emit_partials(2, 4)
        if DEBUG_DUMP:
            nc.sync.dma_start(dbg["dbg_qT"][:], qT[:])
            nc.sync.dma_start(dbg["dbg_kTz"][:], kTz[:])
            nc.sync.dma_start(dbg["dbg_kaug"][:], k_aug[:])
            nc.sync.dma_start(dbg["dbg_lipv"][:], lip_vT[:])
            nc.sync.dma_start(dbg["dbg_qs"][:], qs[:])
            nc.sync.dma_start(dbg["dbg_zT"][:], zT[:])
            nc.sync.dma_start(dbg["dbg_avattn"][:], av_attn[:])

        # ---- output finals: out[nb] = partial_sb[nb] + z[:,3] @ Wc[3] ----
        # the partial re-enters through the PE (K=128 identity matmul
        # accumulating into the same psum), so the tail needs only one
        # psum->sbuf evacuation per [128,1024] (alternating ACT/DVE) + DMA.
        d_out_r = d_out[:].rearrange("(nb p) c -> nb p c", p=128)
        with tc.tile_pool(name="outp", bufs=3) as outp:
            for i in range(4):
                ps = psA.tile([128, N], f32, tag="psA")
                for half in range(2):
                    nb = 2 * i + half
                    sl = slice(half * 512, (half + 1) * 512)
                    nc.tensor.matmul(
                        ps[:, sl],
                        zT[:, CB - 1, nb * 128:(nb + 1) * 128],
                        Wc_sb[:, CB - 1, :],
                        start=True, stop=False,
                    )
                    nc.tensor.matmul(
                        ps[:, sl], ident_sb[:], partial_sb[:, 2 * i + half, :],
                        start=False, stop=True,
                    )
                o2 = outp.tile([128, 2, C], bf16, tag="o_sb")
                if i % 2 == 0:
                    nc.scalar.activation(o2[:], ps[:], AF.Copy)
                else:
                    nc.vector.tensor_copy(out=o2[:], in_=ps[:])
                for half in range(2):
                    nb = 2 * i + half
                    # drain the stores over the scalar+sync DMA rings only:
                    # a gpsimd-ring store here costs a ~2.3us gpsimd pipe
                    # DRAIN in the postamble, after everything else is done
                    if nb % 2 == 0:
                        nc.scalar.dma_start(d_out_r[nb], o2[:, half])
                    else:
                        nc.sync.dma_start(d_out_r[nb], o2[:, half])

    nc.compile()
    return nc


def _marshal(audia, lip, W_qkv, W_lip, W_proj, b_proj, W_se):
    bf16 = ml_dtypes.bfloat16
    WqkT = np.ascontiguousarray(W_qkv[:2 * C].T.astype(bf16))
    WlipT = np.ascontiguousarray(W_lip.T.astype(bf16))
    WseT = np.ascontiguousarray(W_se.T.astype(bf16))
    WprojN = np.ascontiguousarray(W_proj.astype(bf16))
    bprojP = np.zeros((128, CB, HD + 1), bf16)
    bprojP[:, :, 0] = np.asarray(b_proj, np.float32).reshape(CB, 128).T.astype(bf16)
    ident = np.eye(128, dtype=np.float32).astype(bf16)
    sel = np.zeros((128, HD + 1), bf16)
    sel[0:64, 0] = 1
    sel[64:128, HD] = 1
    selC = np.zeros((HD + 1, 128), bf16)
    selC[0, 0:64] = 1
    selC[HD, 64:128] = 1
    onesK = np.zeros((HD + 1, 128), bf16)
    onesK[0, :] = 1
    in_maps = []
    for b in range(B):
        in_maps.append({
            "audiaT": np.ascontiguousarray(audia[b].T.astype(bf16)),
            "lipT": np.ascontiguousarray(lip[b].T.astype(bf16)),
            "WqkT": WqkT, "WlipT": WlipT, "WseT": WseT, "WprojN": WprojN,
            "bprojP": bprojP, "ident": ident, "sel": sel, "selC": selC,
            "onesK": onesK,
        })
    return in_maps


def run(inputs, trace=False, **kw):
    from concourse.bass_utils import run_bass_kernel_spmd
    if "nc" not in _CACHED:
        _CACHED["nc"] = build_nc()
    in_maps = _marshal(**inputs)
    return run_bass_kernel_spmd(
        _CACHED["nc"], in_maps, core_ids=list(range(B)), trace=trace, **kw
    )


def kernel(audia, lip, W_qkv, W_lip, W_proj, b_proj, W_se):
    res = run(dict(audia=audia, lip=lip, W_qkv=W_qkv, W_lip=W_lip,
                   W_proj=W_proj, b_proj=b_proj, W_se=W_se))
    return np.stack([r["out"] for r in res.results], 0).astype(np.float32)

